# revision 8
# baseline (speedup 1.0000x reference)
"""Single-head attention (B=4, S=2048, E=1024, fp32) on 8 trn2 NeuronCores.

Sharding: (batch, q-half) -> 8 shards. Core c handles batch c//2, query rows
[h*1024, (h+1)*1024) with h = c%2. Each core computes K/V projections for the
full 2048-row sequence of its batch (duplicated within the pair), its own Q
half, scores^T, softmax (no max subtraction -- scores are O(1) here), and the
output rows.

Device kernel layouts (per core):
  xt  [E, S]   x[b].T with the core's q-half columns permuted first
               (softmax/output are invariant to key order, so K/V may use the
               permuted order as long as it is consistent).
  QT  [f, q]   f on partitions -> scores contraction over f needs this.
  KT  [f, s]   same.
  S^T [k, q]   k on partitions -> rowsum via matmul with ones, O uses P^T
               directly as the stationary operand.
  V   [s, f]   natural layout, moving operand of the O matmul.

P^T = exp(S^T) is bounced through DRAM ([k_tile, q_tile, 128, 128] tiles) so
SBUF pool lifetimes nest: {xt,qt,kt} die before {wvt,v} are allocated.

All matmuls run as float32r (full fp32 data, 1 cycle/row on the PE for moving
dim >= 256).
"""

import numpy as np

P = 128


def _emit(nc, E=1024, S=2048, SQ=1024, SB=256):
    """Emit the per-core kernel IR into `nc`."""
    import concourse.mybir as mybir
    import concourse.tile as tile

    f32 = mybir.dt.float32
    f32r = mybir.dt.float32r
    ACT = mybir.ActivationFunctionType

    ET = E // P          # e/f tiles (8)
    ST = S // P          # s/k tiles (16)
    QTN = SQ // P        # q tiles (8)
    NQC = SQ // 512      # q chunks of 512 (2)
    NFC = E // 512       # f chunks of 512 (2)
    NSB = SB // P        # s-subtiles per V stationary block (2)

    xt = nc.dram_tensor("xt", [E, S], f32r, kind="ExternalInput")
    xv = nc.dram_tensor("xv", [ET, S // SB, P, SB], f32r, kind="ExternalInput")
    wq4 = nc.dram_tensor("wq4", [ET, ET, P, P], f32r, kind="ExternalInput")
    wk4 = nc.dram_tensor("wk4", [ET, ET, P, P], f32r, kind="ExternalInput")
    wvt = nc.dram_tensor("wvt", [E, E], f32r, kind="ExternalInput")
    bq8 = nc.dram_tensor("bq8", [P, ET], f32, kind="ExternalInput")
    bk8 = nc.dram_tensor("bk8", [P, ET], f32, kind="ExternalInput")
    bvb = nc.dram_tensor("bvb", [P, E], f32, kind="ExternalInput")
    ones2 = nc.dram_tensor("ones2", [P, 2], f32r, kind="ExternalInput")
    o = nc.dram_tensor("o", [SQ, E], f32, kind="ExternalOutput")

    with tile.TileContext(nc) as tc:
        dram_cm = tc.tile_pool(name="dramp", bufs=1, space="DRAM")
        dramp = dram_cm.__enter__()
        ptd = dramp.tile([ST, QTN, P, P], f32r, tag="ptd")
        psum_cm = tc.tile_pool(name="psum", bufs=6, space="PSUM")
        psum = psum_cm.__enter__()
        rs_cm = tc.tile_pool(name="rspsum", bufs=2, space="PSUM")
        rsp = rs_cm.__enter__()
        small_cm = tc.tile_pool(name="small", bufs=1)
        small = small_cm.__enter__()

        qk_cm = tc.tile_pool(name="qk", bufs=1)
        qk = qk_cm.__enter__()
        qt_t = qk.tile([P, ET, SQ], f32r, tag="qt")
        kt_t = qk.tile([P, ET, S], f32r, tag="kt")

        # bias tiles up front (they live in `small` for the whole kernel)
        bq_t = small.tile([P, ET], f32, tag="bq")
        nc.sync.dma_start(bq_t[:], bq8[:])
        bk_t = small.tile([P, ET], f32, tag="bk")
        nc.sync.dma_start(bk_t[:], bk8[:])
        bv_t = small.tile([P, E], f32, tag="bv")
        nc.sync.dma_start(bv_t[:], bvb[:])
        ones_t = small.tile([P, 2], f32r, tag="ones")
        nc.sync.dma_start(ones_t[:], ones2[:])

        # ---------------- phase 1: QT and KT projections ----------------
        xt_cm = tc.tile_pool(name="xtp", bufs=1)
        xtp = xt_cm.__enter__()
        w_cm = tc.tile_pool(name="wstream", bufs=4)
        wsp = w_cm.__enter__()

        xt_t = xtp.tile([P, ET, S], f32r, tag="xt")
        for e in range(ET):
            nc.sync.dma_start(xt_t[:, e], xt[e * P : (e + 1) * P, :])

        for w4, bias_t, out_t, ncols in (
            (wq4, bq_t, qt_t, SQ),
            (wk4, bk_t, kt_t, S),
        ):
            ncc = ncols // 512
            for f in range(ET):
                ps = [
                    psum.tile([P, 512], f32, tag="mm", name=f"ps{j}")
                    for j in range(ncc)
                ]
                for e in range(ET):
                    w_t = wsp.tile([P, P], f32r, tag="w")
                    nc.sync.dma_start(w_t[:], w4[e, f])
                    for j in range(ncc):
                        nc.tensor.matmul(
                            ps[j][:],
                            w_t[:],
                            xt_t[:, e, j * 512 : (j + 1) * 512],
                            start=(e == 0),
                            stop=(e == ET - 1),
                        )
                for j in range(ncc):
                    nc.scalar.add(
                        out_t[:, f, j * 512 : (j + 1) * 512],
                        ps[j][:],
                        bias_t[:, f : f + 1],
                    )
        w_cm.__exit__(None, None, None)
        xt_cm.__exit__(None, None, None)

        # ---------------- phase 2: scores^T + exp -> PT (to DRAM) ----------------
        exp_cm = tc.tile_pool(name="expp", bufs=4)
        expp = exp_cm.__enter__()
        for k in range(ST):
            ps = [
                psum.tile([P, 512], f32, tag="mm", name=f"ps{j}")
                for j in range(NQC)
            ]
            for f in range(ET):
                for qc in range(NQC):
                    nc.tensor.matmul(
                        ps[qc][:],
                        kt_t[:, f, k * P : (k + 1) * P],
                        qt_t[:, f, qc * 512 : (qc + 1) * 512],
                        start=(f == 0),
                        stop=(f == ET - 1),
                    )
            for qc in range(NQC):
                e_t = expp.tile([P, 512], f32r, tag="exp")
                nc.scalar.activation(e_t[:], ps[qc][:], ACT.Exp)
                nqt = 512 // P  # q tiles per chunk (4)
                nc.sync.dma_start(
                    ptd[k, qc * nqt : (qc + 1) * nqt].rearrange("t p q -> p t q"),
                    e_t[:].rearrange("p (t q) -> p t q", q=P),
                )
        exp_cm.__exit__(None, None, None)
        qk_cm.__exit__(None, None, None)

        # ---------------- phase 3: V projection ----------------
        v_cm = tc.tile_pool(name="vp", bufs=1)
        vp = v_cm.__enter__()
        xs_cm = tc.tile_pool(name="xstream", bufs=4)
        xsp = xs_cm.__enter__()

        wvt_t = vp.tile([P, ET, E], f32r, tag="wvt")
        for e in range(ET):
            nc.sync.dma_start(wvt_t[:, e], wvt[e * P : (e + 1) * P, :])
        v_t = vp.tile([P, ST, E], f32r, tag="v")

        for sb in range(S // SB):
            ps = [
                [
                    psum.tile([P, 512], f32, tag="mm", name=f"ps{si}_{fc}")
                    for fc in range(NFC)
                ]
                for si in range(NSB)
            ]
            for e in range(ET):
                xv_t = xsp.tile([P, SB], f32r, tag="xv")
                nc.sync.dma_start(xv_t[:], xv[e, sb])
                for si in range(NSB):
                    for fc in range(NFC):
                        nc.tensor.matmul(
                            ps[si][fc][:],
                            xv_t[:, si * P : (si + 1) * P],
                            wvt_t[:, e, fc * 512 : (fc + 1) * 512],
                            start=(e == 0),
                            stop=(e == ET - 1),
                        )
            for si in range(NSB):
                st = sb * NSB + si
                for fc in range(NFC):
                    nc.vector.tensor_add(
                        v_t[:, st, fc * 512 : (fc + 1) * 512],
                        ps[si][fc][:],
                        bv_t[:, fc * 512 : (fc + 1) * 512],
                    )
        xs_cm.__exit__(None, None, None)

        # ---------------- phase 4: O = softmax-normalized P^T.T @ V ----------------
        pts_cm = tc.tile_pool(name="pts", bufs=6)
        pts = pts_cm.__enter__()
        ob_cm = tc.tile_pool(name="ob", bufs=3)
        obp = ob_cm.__enter__()
        for qt_i in range(QTN):
            po = [
                psum.tile([P, 512], f32, tag="mm", name=f"po{j}")
                for j in range(NFC)
            ]
            prs = rsp.tile([P, 2], f32, tag="rs")
            for k in range(ST):
                pt_t = pts.tile([P, P], f32r, tag="pt")
                nc.sync.dma_start(pt_t[:], ptd[k, qt_i])
                lhs = pt_t[:]
                for fc in range(NFC):
                    nc.tensor.matmul(
                        po[fc][:],
                        lhs,
                        v_t[:, k, fc * 512 : (fc + 1) * 512],
                        start=(k == 0),
                        stop=(k == ST - 1),
                    )
                nc.tensor.matmul(
                    prs[:],
                    lhs,
                    ones_t[:],
                    start=(k == 0),
                    stop=(k == ST - 1),
                )
            recip = obp.tile([P, 1], f32, tag="recip")
            nc.vector.reciprocal(recip[:], prs[:, 0:1])
            o_t = obp.tile([P, E], f32, tag="ob")
            for fc in range(NFC):
                nc.vector.tensor_scalar_mul(
                    o_t[:, fc * 512 : (fc + 1) * 512], po[fc][:], recip[:]
                )
            nc.sync.dma_start(o[qt_i * P : (qt_i + 1) * P, :], o_t[:])
        ob_cm.__exit__(None, None, None)
        pts_cm.__exit__(None, None, None)

        v_cm.__exit__(None, None, None)
        small_cm.__exit__(None, None, None)
        rs_cm.__exit__(None, None, None)
        psum_cm.__exit__(None, None, None)
        dram_cm.__exit__(None, None, None)


_NC_CACHE = {}


def build_nc(E=1024, S=2048, SQ=1024, SB=256):
    key = (E, S, SQ, SB)
    if key in _NC_CACHE:
        return _NC_CACHE[key]
    import concourse.bacc as bacc

    nc = bacc.Bacc(None, target_bir_lowering=False)
    _emit(nc, E=E, S=S, SQ=SQ, SB=SB)
    nc.finalize()
    _NC_CACHE[key] = nc
    return nc


def _round_f32r(a):
    """Round fp32 to fp32r (tf32-like: 11 explicit mantissa bits, RNE)."""
    u = np.ascontiguousarray(a, np.float32).view(np.uint32)
    u = u + np.uint32(0x7FF) + ((u >> np.uint32(12)) & np.uint32(1))
    return (u & np.uint32(0xFFFFF000)).view(np.float32)


def make_in_maps(x, Wq, bq, Wk, bk, Wv, bv, E=1024, S=2048, SQ=1024, SB=256):
    """Host-side prep: per-core input dicts for run_bass_kernel_spmd."""
    ET = E // P
    scale = 1.0 / np.sqrt(np.float32(E))
    x = np.asarray(x, np.float32)
    B = x.shape[0]
    n_half = S // SQ

    # Weight tiles [e_tile, f_tile, p, f] so each stationary DMA is contiguous.
    def tile4(wt):  # wt: [E, E] (e rows, f cols)
        return np.ascontiguousarray(wt.reshape(ET, P, ET, P).transpose(0, 2, 1, 3))

    wq4 = _round_f32r(tile4(np.asarray(Wq, np.float32).T * scale))
    wk4 = _round_f32r(tile4(np.asarray(Wk, np.float32).T))
    wvt_h = _round_f32r(np.ascontiguousarray(np.asarray(Wv, np.float32).T))
    bq8 = np.ascontiguousarray((np.asarray(bq, np.float32) * scale).reshape(ET, P).T)
    bk8 = np.ascontiguousarray(np.asarray(bk, np.float32).reshape(ET, P).T)
    bvb = np.ascontiguousarray(np.broadcast_to(np.asarray(bv, np.float32), (P, E)))

    in_maps = []
    for c in range(B * n_half):
        b, h = divmod(c, n_half)
        xt_full = x[b].T  # [E, S]
        order = [h] + [i for i in range(n_half) if i != h]
        xt_perm = _round_f32r(
            np.concatenate([xt_full[:, i * SQ : (i + 1) * SQ] for i in order], axis=1)
        )
        xv = np.ascontiguousarray(
            xt_perm.reshape(ET, P, S // SB, SB).transpose(0, 2, 1, 3)
        )
        in_maps.append(
            {
                "ones2": np.ones((P, 2), np.float32),
                "xt": xt_perm,
                "xv": xv,
                "wq4": wq4,
                "wk4": wk4,
                "wvt": wvt_h,
                "bq8": bq8,
                "bk8": bk8,
                "bvb": bvb,
            }
        )
    return in_maps


def kernel(x, Wq, bq, Wk, bk, Wv, bv):
    from concourse.bass_utils import run_bass_kernel_spmd

    E, S, SQ = 1024, 2048, 1024
    x = np.asarray(x, np.float32)
    B = x.shape[0]
    nc = build_nc(E=E, S=S, SQ=SQ)
    in_maps = make_in_maps(x, Wq, bq, Wk, bk, Wv, bv, E=E, S=S, SQ=SQ)
    n_cores = len(in_maps)
    res = run_bass_kernel_spmd(nc, in_maps, list(range(n_cores)))
    out = np.empty((B, S, E), np.float32)
    n_half = S // SQ
    for c in range(n_cores):
        b, h = divmod(c, n_half)
        out[b, h * SQ : (h + 1) * SQ, :] = res.results[c]["o"]
    return out


# revision 9
# speedup vs baseline: 1.1277x; 1.1277x over previous
"""Single-head attention (B=4, S=2048, E=1024, fp32) on 8 trn2 NeuronCores.

Sharding: (batch, q-half) -> 8 shards. Core c handles batch c//2, query rows
[h*1024, (h+1)*1024) with h = c%2. Each core computes K/V projections for the
full 2048-row sequence of its batch (duplicated within the pair), its own Q
half, scores^T, softmax (no max subtraction -- scores are O(1) here), and the
output rows.

Device kernel layouts (per core):
  xt  [E, S]   x[b].T with the core's q-half columns permuted first
               (softmax/output are invariant to key order, so K/V may use the
               permuted order as long as it is consistent).
  QT  [f, q]   f on partitions -> scores contraction over f needs this.
  KT  [f, s]   same.
  S^T [k, q]   k on partitions -> rowsum via matmul with ones, O uses P^T
               directly as the stationary operand.
  V   [s, f]   natural layout, moving operand of the O matmul.

P^T = exp(S^T) is bounced through DRAM ([k_tile, q_tile, 128, 128] tiles) so
SBUF pool lifetimes nest: {xt,qt,kt} die before {wvt,v} are allocated.

All matmuls run as float32r (full fp32 data, 1 cycle/row on the PE for moving
dim >= 256).
"""

import numpy as np

P = 128


def _emit(nc, E=1024, S=2048, SQ=1024, SB=256):
    """Emit the per-core kernel IR into `nc`."""
    import concourse.mybir as mybir
    import concourse.tile as tile

    f32 = mybir.dt.float32
    f32r = mybir.dt.float32r
    ACT = mybir.ActivationFunctionType

    ET = E // P          # e/f tiles (8)
    ST = S // P          # s/k tiles (16)
    QTN = SQ // P        # q tiles (8)
    NQC = SQ // 512      # q chunks of 512 (2)
    NFC = E // 512       # f chunks of 512 (2)
    NSB = SB // P        # s-subtiles per V stationary block (2)

    xt = nc.dram_tensor("xt", [E, S], f32r, kind="ExternalInput")
    xv = nc.dram_tensor("xv", [S // SB, ET, P, SB], f32r, kind="ExternalInput")  # [sb,e,p,c]
    wq4 = nc.dram_tensor("wq4", [ET, ET, P, P], f32r, kind="ExternalInput")  # [f,e,p,c]
    wk4 = nc.dram_tensor("wk4", [ET, ET, P, P], f32r, kind="ExternalInput")  # [f,e,p,c]
    wvt = nc.dram_tensor("wvt", [E, E], f32r, kind="ExternalInput")
    bq8 = nc.dram_tensor("bq8", [P, ET], f32, kind="ExternalInput")
    bk8 = nc.dram_tensor("bk8", [P, ET], f32, kind="ExternalInput")
    bvb = nc.dram_tensor("bvb", [P, E], f32, kind="ExternalInput")
    ones2 = nc.dram_tensor("ones2", [P, 2], f32r, kind="ExternalInput")
    o = nc.dram_tensor("o", [SQ, E], f32, kind="ExternalOutput")

    with tile.TileContext(nc) as tc:
        dram_cm = tc.tile_pool(name="dramp", bufs=1, space="DRAM")
        dramp = dram_cm.__enter__()
        ptd = dramp.tile([QTN, ST, P, P], f32r, tag="ptd")
        psum_cm = tc.tile_pool(name="psum", bufs=6, space="PSUM")
        psum = psum_cm.__enter__()
        rs_cm = tc.tile_pool(name="rspsum", bufs=2, space="PSUM")
        rsp = rs_cm.__enter__()
        small_cm = tc.tile_pool(name="small", bufs=1)
        small = small_cm.__enter__()

        qk_cm = tc.tile_pool(name="qk", bufs=1)
        qk = qk_cm.__enter__()
        qt_t = qk.tile([P, ET, SQ], f32r, tag="qt")
        kt_t = qk.tile([P, ET, S], f32r, tag="kt")

        # bias tiles up front (they live in `small` for the whole kernel)
        bq_t = small.tile([P, ET], f32, tag="bq")
        nc.sync.dma_start(bq_t[:], bq8[:])
        bk_t = small.tile([P, ET], f32, tag="bk")
        nc.sync.dma_start(bk_t[:], bk8[:])
        bv_t = small.tile([P, E], f32, tag="bv")
        nc.sync.dma_start(bv_t[:], bvb[:])
        ones_t = small.tile([P, 2], f32r, tag="ones")
        nc.sync.dma_start(ones_t[:], ones2[:])

        # ---------------- phase 1: QT and KT projections ----------------
        xt_cm = tc.tile_pool(name="xtp", bufs=1)
        xtp = xt_cm.__enter__()
        w_cm = tc.tile_pool(name="wstream", bufs=4)
        wsp = w_cm.__enter__()

        xt_t = xtp.tile([P, ET, S], f32r, tag="xt")
        for e in range(ET):
            nc.sync.dma_start(xt_t[:, e], xt[e * P : (e + 1) * P, :])

        for w4, bias_t, out_t, ncols in (
            (wq4, bq_t, qt_t, SQ),
            (wk4, bk_t, kt_t, S),
        ):
            ncc = ncols // 512
            for f in range(ET):
                ps = [
                    psum.tile([P, 512], f32, tag="mm", name=f"ps{j}")
                    for j in range(ncc)
                ]
                w_t = wsp.tile([P, ET, P], f32r, tag="w")
                nc.sync.dma_start(w_t[:], w4[f].rearrange("e p c -> p e c"))
                for e in range(ET):
                    for j in range(ncc):
                        nc.tensor.matmul(
                            ps[j][:],
                            w_t[:, e],
                            xt_t[:, e, j * 512 : (j + 1) * 512],
                            start=(e == 0),
                            stop=(e == ET - 1),
                        )
                for j in range(ncc):
                    nc.scalar.add(
                        out_t[:, f, j * 512 : (j + 1) * 512],
                        ps[j][:],
                        bias_t[:, f : f + 1],
                    )
        w_cm.__exit__(None, None, None)
        xt_cm.__exit__(None, None, None)

        # ---------------- phase 2: scores^T + exp -> PT (to DRAM) ----------------
        exp_cm = tc.tile_pool(name="expp", bufs=4)
        expp = exp_cm.__enter__()
        for k in range(ST):
            ps = [
                psum.tile([P, 512], f32, tag="mm", name=f"ps{j}")
                for j in range(NQC)
            ]
            for f in range(ET):
                for qc in range(NQC):
                    nc.tensor.matmul(
                        ps[qc][:],
                        kt_t[:, f, k * P : (k + 1) * P],
                        qt_t[:, f, qc * 512 : (qc + 1) * 512],
                        start=(f == 0),
                        stop=(f == ET - 1),
                    )
            for qc in range(NQC):
                e_t = expp.tile([P, 512], f32r, tag="exp")
                nc.scalar.activation(e_t[:], ps[qc][:], ACT.Exp)
                nqt = 512 // P  # q tiles per chunk (4)
                nc.sync.dma_start(
                    ptd[qc * nqt : (qc + 1) * nqt, k].rearrange("t p q -> p t q"),
                    e_t[:].rearrange("p (t q) -> p t q", q=P),
                )
        exp_cm.__exit__(None, None, None)
        qk_cm.__exit__(None, None, None)

        # ---------------- phase 3: V projection ----------------
        v_cm = tc.tile_pool(name="vp", bufs=1)
        vp = v_cm.__enter__()
        xs_cm = tc.tile_pool(name="xstream", bufs=4)
        xsp = xs_cm.__enter__()

        wvt_t = vp.tile([P, ET, E], f32r, tag="wvt")
        for e in range(ET):
            nc.sync.dma_start(wvt_t[:, e], wvt[e * P : (e + 1) * P, :])
        v_t = vp.tile([P, ST, E], f32r, tag="v")

        for sb in range(S // SB):
            ps = [
                [
                    psum.tile([P, 512], f32, tag="mm", name=f"ps{si}_{fc}")
                    for fc in range(NFC)
                ]
                for si in range(NSB)
            ]
            xv_t = xsp.tile([P, ET, SB], f32r, tag="xv")
            nc.sync.dma_start(xv_t[:], xv[sb].rearrange("e p c -> p e c"))
            for e in range(ET):
                for si in range(NSB):
                    for fc in range(NFC):
                        nc.tensor.matmul(
                            ps[si][fc][:],
                            xv_t[:, e, si * P : (si + 1) * P],
                            wvt_t[:, e, fc * 512 : (fc + 1) * 512],
                            start=(e == 0),
                            stop=(e == ET - 1),
                        )
            for si in range(NSB):
                st = sb * NSB + si
                for fc in range(NFC):
                    nc.vector.tensor_add(
                        v_t[:, st, fc * 512 : (fc + 1) * 512],
                        ps[si][fc][:],
                        bv_t[:, fc * 512 : (fc + 1) * 512],
                    )
        xs_cm.__exit__(None, None, None)

        # ---------------- phase 4: O = softmax-normalized P^T.T @ V ----------------
        pts_cm = tc.tile_pool(name="pts", bufs=3)
        pts = pts_cm.__enter__()
        ob_cm = tc.tile_pool(name="ob", bufs=3)
        obp = ob_cm.__enter__()
        for qt_i in range(QTN):
            po = [
                psum.tile([P, 512], f32, tag="mm", name=f"po{j}")
                for j in range(NFC)
            ]
            prs = rsp.tile([P, 2], f32, tag="rs")
            pt_t = pts.tile([P, ST, P], f32r, tag="pt")
            nc.sync.dma_start(pt_t[:], ptd[qt_i].rearrange("k p q -> p k q"))
            for k in range(ST):
                lhs = pt_t[:, k]
                for fc in range(NFC):
                    nc.tensor.matmul(
                        po[fc][:],
                        lhs,
                        v_t[:, k, fc * 512 : (fc + 1) * 512],
                        start=(k == 0),
                        stop=(k == ST - 1),
                    )
                nc.tensor.matmul(
                    prs[:],
                    lhs,
                    ones_t[:],
                    start=(k == 0),
                    stop=(k == ST - 1),
                )
            recip = obp.tile([P, 1], f32, tag="recip")
            nc.vector.reciprocal(recip[:], prs[:, 0:1])
            o_t = obp.tile([P, E], f32, tag="ob")
            for fc in range(NFC):
                nc.vector.tensor_scalar_mul(
                    o_t[:, fc * 512 : (fc + 1) * 512], po[fc][:], recip[:]
                )
            nc.sync.dma_start(o[qt_i * P : (qt_i + 1) * P, :], o_t[:])
        ob_cm.__exit__(None, None, None)
        pts_cm.__exit__(None, None, None)

        v_cm.__exit__(None, None, None)
        small_cm.__exit__(None, None, None)
        rs_cm.__exit__(None, None, None)
        psum_cm.__exit__(None, None, None)
        dram_cm.__exit__(None, None, None)


_NC_CACHE = {}


def build_nc(E=1024, S=2048, SQ=1024, SB=256):
    key = (E, S, SQ, SB)
    if key in _NC_CACHE:
        return _NC_CACHE[key]
    import concourse.bacc as bacc

    nc = bacc.Bacc(None, target_bir_lowering=False)
    _emit(nc, E=E, S=S, SQ=SQ, SB=SB)
    nc.finalize()
    _NC_CACHE[key] = nc
    return nc


def _round_f32r(a):
    """Round fp32 to fp32r (tf32-like: 11 explicit mantissa bits, RNE)."""
    u = np.ascontiguousarray(a, np.float32).view(np.uint32)
    u = u + np.uint32(0x7FF) + ((u >> np.uint32(12)) & np.uint32(1))
    return (u & np.uint32(0xFFFFF000)).view(np.float32)


def make_in_maps(x, Wq, bq, Wk, bk, Wv, bv, E=1024, S=2048, SQ=1024, SB=256):
    """Host-side prep: per-core input dicts for run_bass_kernel_spmd."""
    ET = E // P
    scale = 1.0 / np.sqrt(np.float32(E))
    x = np.asarray(x, np.float32)
    B = x.shape[0]
    n_half = S // SQ

    # Weight tiles [e_tile, f_tile, p, f] so each stationary DMA is contiguous.
    def tile4(wt):  # wt: [E, E] (e rows, f cols) -> [f_tile, e_tile, p(e), c(f)]
        return np.ascontiguousarray(wt.reshape(ET, P, ET, P).transpose(2, 0, 1, 3))

    wq4 = _round_f32r(tile4(np.asarray(Wq, np.float32).T * scale))
    wk4 = _round_f32r(tile4(np.asarray(Wk, np.float32).T))
    wvt_h = _round_f32r(np.ascontiguousarray(np.asarray(Wv, np.float32).T))
    bq8 = np.ascontiguousarray((np.asarray(bq, np.float32) * scale).reshape(ET, P).T)
    bk8 = np.ascontiguousarray(np.asarray(bk, np.float32).reshape(ET, P).T)
    bvb = np.ascontiguousarray(np.broadcast_to(np.asarray(bv, np.float32), (P, E)))

    in_maps = []
    for c in range(B * n_half):
        b, h = divmod(c, n_half)
        xt_full = x[b].T  # [E, S]
        order = [h] + [i for i in range(n_half) if i != h]
        xt_perm = _round_f32r(
            np.concatenate([xt_full[:, i * SQ : (i + 1) * SQ] for i in order], axis=1)
        )
        xv = np.ascontiguousarray(
            xt_perm.reshape(ET, P, S // SB, SB).transpose(2, 0, 1, 3)
        )
        in_maps.append(
            {
                "ones2": np.ones((P, 2), np.float32),
                "xt": xt_perm,
                "xv": xv,
                "wq4": wq4,
                "wk4": wk4,
                "wvt": wvt_h,
                "bq8": bq8,
                "bk8": bk8,
                "bvb": bvb,
            }
        )
    return in_maps


def kernel(x, Wq, bq, Wk, bk, Wv, bv):
    from concourse.bass_utils import run_bass_kernel_spmd

    E, S, SQ = 1024, 2048, 1024
    x = np.asarray(x, np.float32)
    B = x.shape[0]
    nc = build_nc(E=E, S=S, SQ=SQ)
    in_maps = make_in_maps(x, Wq, bq, Wk, bk, Wv, bv, E=E, S=S, SQ=SQ)
    n_cores = len(in_maps)
    res = run_bass_kernel_spmd(nc, in_maps, list(range(n_cores)))
    out = np.empty((B, S, E), np.float32)
    n_half = S // SQ
    for c in range(n_cores):
        b, h = divmod(c, n_half)
        out[b, h * SQ : (h + 1) * SQ, :] = res.results[c]["o"]
    return out
